# revision 39
# baseline (speedup 1.0000x reference)
"""Trainium2 Bass kernel for nn_AdaptiveMCTSReasoner.

Self-contained: kernel(**inputs) takes FULL inputs, shards batch across 8
NeuronCores (2 examples/core), runs one Bass/Tile kernel per core, gathers.

Algorithm restructuring (validated vs reference, rel err 4.6e-7 in fp32):
  - the 100-step scan is independent given root:
      acc = (1 + sum_t w_t) * root + sum_{t,j} w_t * (new_{t,j} - root[pos_{t,j}])
    scattered at <=300 rows/example (one-hot matmul scatter).
  - transition MLPs batch over all 100 steps (padded to 128 cols/example).
  - policy L2 mean folds into a vector: focus = GELU(root@Wp1+b) @ rowmean(Wp2).
  - gumbel noise is data-independent (key 42) -> precomputed on host exactly.
Layout: activations feature-major [H on partitions, tokens on free] so weights
load as natural [h_in, h_out] lhsT tiles (no weight transposes).
Precision: bf16 matmuls for bulk stages (error budget ~5e-3 << 2e-2 gate),
fp32 for the sim-controller head (its argmax gap is 2e-4 on this data),
float32r for the aggregation second layer.
"""
import numpy as np

B, S, H = 16, 1024, 1024
T, KF = 100, 3
NCORES = 8
BL = B // NCORES          # 2 examples per core
TOK = BL * S              # 2048 tokens per core
TP = 128                  # padded step dim per example
NT = BL * TP              # 256 step-columns per core
HK = H // 128             # 8 feature tiles

_CACHE = {}
DEBUG = False


def _install_patches(mybir, TileContext, ScopedClock):
    """This walrus build allows ONE sync wait / update per instruction.
    Split excess waits onto standalone InstEventSemaphore instructions at the
    same program position on the same engine queue (semantics preserving)."""
    if getattr(TileContext, "_mcts_patched", False):
        return
    _orig_lower = TileContext._lower_ordered_insts
    counter = [0]

    def _is_async(inst):
        n = type(inst).__name__
        return n.startswith("InstDMA") or "Collective" in n

    def _mk_event(engine, waits, updates):
        counter[0] += 1
        ev = mybir.InstEventSemaphore(name=f"I-wsplit-{counter[0]}", ins=[], outs=[])
        ev.engine = engine
        ev.sync_info = mybir.SyncInfo(on_wait=list(waits), on_update=list(updates))
        return ev

    def _patched_lower(self, ordered):
        for bb_name, insts in list(ordered.items()):
            new_insts = []
            for inst in insts:
                si = inst.sync_info
                waits = list(si.on_wait) if si else []
                ups = list(si.on_update) if si else []
                changed = False
                if len(waits) > 1:
                    for w in waits[:-1]:
                        new_insts.append(_mk_event(inst.engine, [w], []))
                    waits = [waits[-1]]
                    changed = True
                post = []
                if len(ups) > 1 and not _is_async(inst):
                    for u in ups[1:]:
                        post.append(_mk_event(inst.engine, [], [u]))
                    ups = [ups[0]]
                    changed = True
                if changed:
                    inst.sync_info = mybir.SyncInfo(on_wait=waits, on_update=ups)
                new_insts.append(inst)
                new_insts.extend(post)
            ordered[bb_name] = new_insts
        return _orig_lower(self, ordered)

    def _patched_drain(self, tick_clock, wait_clock):
        drain_inst = self.nc.sync.drain()
        wait_clock.add_sem_waits(drain_inst.ins, ScopedClock({None: tick_clock.global_clock}))
        waits = list(drain_inst.ins.sync_info.on_wait)
        if len(waits) > 1:
            drain_inst.ins.sync_info = mybir.SyncInfo(on_wait=waits[:1], on_update=[])
            for i in range(1, len(waits)):
                extra = self.nc.sync.drain()
                extra.ins.sync_info = mybir.SyncInfo(on_wait=[waits[i]], on_update=[])
        self.nc.all_engine_barrier()
        popped = self.nc._tile_sem_poison_stack.pop()
        assert popped is self._sem_poison
        self.nc.clear_and_free_semaphores(list(self.sems.allocated().values()))
        self.nc.all_engine_barrier()

    TileContext._lower_ordered_insts = _patched_lower
    TileContext._drain_and_barrier = _patched_drain
    TileContext._mcts_patched = True


def build_nc():
    from contextlib import ExitStack
    from concourse import bass, mybir
    from concourse.tile import TileContext
    from concourse.vector_clock import ScopedClock
    from concourse.masks import make_identity

    _install_patches(mybir, TileContext, ScopedClock)

    f32 = mybir.dt.float32
    f32r = mybir.dt.float32r
    bf16 = mybir.dt.bfloat16
    i32 = mybir.dt.int32
    u32 = mybir.dt.uint32
    AF = mybir.ActivationFunctionType
    OP = mybir.AluOpType
    AX = mybir.AxisListType
    Gelu, Sigm, Ident = AF.Gelu, AF.Sigmoid, AF.Identity

    nc = bass.Bass()
    dp = nc.declare_dram_parameter
    hs = dp("hs", [TOK, H], f32, isOutput=False)
    hs_bf = dp("hs_bf", [TOK, H], bf16, isOutput=False)
    hs_bfT = dp("hs_bfT", [H, TOK], bf16, isOutput=False)     # host-transposed
    mask = dp("mask", [1, TOK], i32, isOutput=False)
    g_in = dp("g", [BL, TP, S], f32, isOutput=False)          # gumbel, t>=100 rows = 0
    simopt = dp("simopt", [5, 1], i32, isOutput=False)
    wp2m_in = dp("wp2m_in", [H], f32, isOutput=False)         # rowmean(Wp2) (host)
    bp2m_in = dp("bp2m_in", [1], f32, isOutput=False)         # mean(bp2) (host)        # [10,25,50,75,100]
    Wsc1 = dp("Wsc1", [H, H], f32, isOutput=False)
    bsc1 = dp("bsc1", [H], f32, isOutput=False)
    Wsc2 = dp("Wsc2", [H, 5], f32, isOutput=False)
    bsc2 = dp("bsc2", [5], f32, isOutput=False)
    Wp1 = dp("Wp1", [H, H], bf16, isOutput=False)
    bp1 = dp("bp1", [H], f32, isOutput=False)
    Wp2 = dp("Wp2", [H, H], f32, isOutput=False)
    bp2 = dp("bp2", [H], f32, isOutput=False)
    Wt1 = dp("Wt1", [2 * H, H], bf16, isOutput=False)
    bt1 = dp("bt1", [H], f32, isOutput=False)
    Wt2 = dp("Wt2", [H, H], bf16, isOutput=False)
    bt2 = dp("bt2", [H], f32, isOutput=False)
    Wav1 = dp("Wav1", [2 * H, H], bf16, isOutput=False)
    bav1 = dp("bav1", [H], f32, isOutput=False)
    Wav2 = dp("Wav2", [H, 1], bf16, isOutput=False)
    bav2 = dp("bav2", [1], f32, isOutput=False)
    Wg1 = dp("Wg1", [2 * H, H], bf16, isOutput=False)
    bg1 = dp("bg1", [H], f32, isOutput=False)
    Wg2 = dp("Wg2", [H, H], bf16, isOutput=False)
    bg2 = dp("bg2", [H], f32, isOutput=False)
    out = dp("out", [TOK, H], f32, isOutput=True)
    if DEBUG:
        dbg_root = dp("dbg_root", [128, TOK], f32, isOutput=True)
        dbg_focus = dp("dbg_focus", [1, TOK], f32, isOutput=True)
        dbg_mi = dp("dbg_mi", [BL, 128, 8], f32, isOutput=True)
        dbg_logits = dp("dbg_logits", [5, BL], f32, isOutput=True)
        dbg_active = dp("dbg_active", [1, NT], f32, isOutput=True)
        dbg_wt = dp("dbg_wt", [1, NT], f32, isOutput=True)
        dbg_crow = dp("dbg_crow", [1, BL], f32, isOutput=True)
        dbg_rm = dp("dbg_rm", [128, 2 * HK], f32, isOutput=True)
        dbg_mean = dp("dbg_mean", [128, NT], f32, isOutput=True)
        dbg_gath = dp("dbg_gath", [128, NT], f32, isOutput=True)
        dbg_diff = dp("dbg_diff", [128, NT], f32, isOutput=True)
        dbg_acc = dp("dbg_acc", [128, 512], f32, isOutput=True)
        dbg_eg = dp("dbg_eg", [128, 512], f32, isOutput=True)

    with TileContext(nc) as tc, ExitStack() as ctx:
        P_ = ctx.enter_context           # pools that live to the end
        const = P_(tc.tile_pool(name="const", bufs=1))
        persist = P_(tc.tile_pool(name="persist", bufs=1))
        bias_p = P_(tc.tile_pool(name="bias", bufs=1))
        small = P_(tc.tile_pool(name="small", bufs=1))
        late = P_(tc.tile_pool(name="late", bufs=1))          # DwT / Pmat (stages 8-9)
        sps = P_(tc.tile_pool(name="sps", bufs=2, space="PSUM"))

        # ---- constants ----
        ident = const.tile([128, 128], f32)
        make_identity(nc, ident[:])
        ones_col = const.tile([1, 128], f32)
        nc.vector.memset(ones_col[:], 1.0)
        iota_f = const.tile([128, S], f32)
        with tc.tile_pool(name="iotp", bufs=1) as iotp:
            iota_i = iotp.tile([128, S], i32, tag="iotai", name="iotai")
            nc.gpsimd.iota(iota_i[:], pattern=[[1, S]], base=0, channel_multiplier=0)
            nc.vector.tensor_copy(iota_f[:], iota_i[:])
        # TmatT[k,t] = (t < simopt[k]), [5, TP]
        iota5 = const.tile([5, TP], i32)
        nc.gpsimd.iota(iota5[:], pattern=[[1, TP]], base=0, channel_multiplier=0)
        iota5f = const.tile([5, TP], f32)
        nc.vector.tensor_copy(iota5f[:], iota5[:])
        so_t = const.tile([5, 1], i32)
        nc.sync.dma_start(so_t[:], simopt[:])
        so_f = const.tile([5, 1], f32)
        nc.vector.tensor_copy(so_f[:], so_t[:])
        TmatT = const.tile([5, TP], f32)
        nc.vector.tensor_scalar(out=TmatT[:], in0=iota5f[:], scalar1=so_f[:, :1],
                                scalar2=None, op0=OP.is_lt)
        ident_bf = const.tile([128, 128], bf16)
        nc.vector.tensor_copy(ident_bf[:], ident[:])
        zero_nt = const.tile([128, TP], f32)
        nc.vector.memset(zero_nt[:], 0.0)

        # ---- bias tiles [128, HK] (col m) per bias vector ----
        def bias_tiles(bvec, name):
            t = bias_p.tile([128, HK], f32, tag=name)
            nc.sync.dma_start(t[:], bvec.rearrange("(m p) -> p m", p=128))
            return t
        bp1_t = bias_tiles(bp1, "bp1")
        bt1_t = bias_tiles(bt1, "bt1")
        bt2_t = bias_tiles(bt2, "bt2")
        bav1_t = bias_tiles(bav1, "bav1")
        bg1_t = bias_tiles(bg1, "bg1")
        bsc2_t = bias_p.tile([5, 1], f32, tag="bsc2")
        nc.sync.dma_start(bsc2_t[:], bsc2[:, None])
        bav2_t = bias_p.tile([1, 1], f32, tag="bav2")
        nc.sync.dma_start(bav2_t[:], bav2[:, None])

        # bg2 broadcast [128, H] (bias along free dim in token-major output)
        bg2_row = small.tile([1, H], f32, tag="bg2row")
        nc.sync.dma_start(bg2_row[:], bg2[None, :])
        bg2_bc = persist.tile([128, H], f32, tag="bg2bc")
        for hc in range(2):
            pstmp = sps.tile([128, 512], f32, space="PSUM", tag="sps")
            nc.tensor.matmul(pstmp[:], ones_col[:], bg2_row[:, hc * 512:(hc + 1) * 512],
                             start=True, stop=True)
            nc.scalar.copy(bg2_bc[:, hc * 512:(hc + 1) * 512], pstmp[:])

        # ---- stage 1: root feature-major via DMA transpose (bf16) ----
        root_bf = [persist.tile([128, TOK], bf16, tag=f"rootbf{k}", name=f"rootbf{k}") for k in range(HK)]
        root0 = small.tile([128, 2 * HK], f32, tag="root0")   # col k*2+b
        with tc.tile_pool(name="r0p", bufs=2) as r0p:
            row_ts = []
            for b in range(BL):
                row_t = r0p.tile([1, H], f32, tag=f"rowt{b}", name=f"rowt{b}")
                nc.sync.dma_start(row_t[:], hs[b * S:b * S + 1, :])
                row_ts.append(row_t)
            for k in range(HK):
                nc.sync.dma_start(root_bf[k][:], hs_bfT[k * 128:(k + 1) * 128, :])
            pst0 = sps.tile([128, 512], f32, space="PSUM", tag="sps")
            for b in range(BL):
                for k in range(HK):
                    nc.tensor.matmul(pst0[:, k * 2 + b:k * 2 + b + 1],
                                     row_ts[b][:, k * 128:(k + 1) * 128],
                                     ones_col[:, 0:1], start=True, stop=True)
            nc.vector.tensor_copy(root0[:], pst0[:, :2 * HK])

        # ---- stage 2: root_mean -> mean0 broadcast tiles (bf16) ----
        rm = small.tile([128, 2 * HK], f32, tag="rm")         # col k*2+b
        for k in range(HK):
            nc.vector.tensor_reduce(rm[:, k * 2:k * 2 + 2],
                                    root_bf[k][:].rearrange("p (b s) -> p b s", b=BL),
                                    axis=AX.X, op=OP.add)
        nc.vector.tensor_scalar(out=rm[:], in0=rm[:], scalar1=1.0 / S, scalar2=None, op0=OP.mult)
        if DEBUG:
            nc.sync.dma_start(dbg_rm[:], rm[:])
        mid = ExitStack()
        meanp = mid.enter_context(tc.tile_pool(name="meanp", bufs=1))
        mean_cur = [meanp.tile([128, NT], bf16, tag=f"mean{k}", name=f"mean{k}") for k in range(HK)]
        mean0 = [meanp.tile([128, NT], bf16, tag=f"mean0{k}", name=f"mean0{k}") for k in range(HK)]
        for k in range(HK):
            for b in range(BL):
                nc.scalar.activation(mean_cur[k][:, b * TP:(b + 1) * TP], zero_nt[:],
                                     Ident, bias=rm[:, k * 2 + b:k * 2 + b + 1], scale=0.0)
        for k in range(HK):
            nc.vector.tensor_copy(mean0[k][:], mean_cur[k][:])

        # ---- stage 3: policy L1 (bf16) + focus logits ----
        focus_row = small.tile([1, TOK], f32, tag="focus")
        wp2m = small.tile([128, HK], bf16, tag="wp2m")
        wp2m_f = small.tile([128, HK], f32, tag="wp2mf")
        nc.sync.dma_start(wp2m_f[:], wp2m_in.rearrange("(m p) -> p m", p=128))
        nc.vector.tensor_copy(wp2m[:], wp2m_f[:])
        bp2m = small.tile([1, 1], f32, tag="bp2m")
        nc.sync.dma_start(bp2m[:], bp2m_in[:, None])
        idxf_b, gidx_b = [], []
        mrow = small.tile([1, TOK], f32, tag="mrow")
        with tc.tile_pool(name="mp", bufs=1) as mp:
            mrow_i = mp.tile([1, TOK], i32, tag="mrowi", name="mrowi")
            nc.sync.dma_start(mrow_i[:], mask[:])
            nc.vector.tensor_copy(mrow[:], mrow_i[:])
            nc.vector.tensor_scalar(out=mrow[:], in0=mrow[:], scalar1=0.0, scalar2=-1e9,
                                    op0=OP.is_equal, op1=OP.mult)
        gp = mid.enter_context(tc.tile_pool(name="gp", bufs=1))
        fbp = mid.enter_context(tc.tile_pool(name="fbp", bufs=2))
        polctx = ExitStack()
        pp = polctx.enter_context(tc.tile_pool(name="pp", bufs=3))
        wp1p = polctx.enter_context(tc.tile_pool(name="wp1p", bufs=1))
        pps = polctx.enter_context(tc.tile_pool(name="pps", bufs=2, space="PSUM"))
        fps = polctx.enter_context(tc.tile_pool(name="fps", bufs=2, space="PSUM"))
        if True:
            wp1_sl = [wp1p.tile([128, H], bf16, tag=f"wp1s{k}", name=f"wp1s{k}") for k in range(HK)]
            for k in range(HK):
                nc.sync.dma_start(wp1_sl[k][:], Wp1[k * 128:(k + 1) * 128, :])
            grow_tiles = {}

            def b_block(b):
                fb = fbp.tile([128, S], f32, tag="fb", name="fb")
                for h in range(2):
                    cs2 = slice(b * S + h * 512, b * S + (h + 1) * 512)
                    nc.vector.tensor_tensor(out=focus_row[:, cs2], in0=focus_row[:, cs2],
                                            in1=mrow[:, cs2], op=OP.add)
                    pstf = sps.tile([128, 512], f32, space="PSUM", tag="sps")
                    nc.tensor.matmul(pstf[:], ones_col[:], focus_row[:, cs2],
                                     start=True, stop=True)
                    nc.scalar.copy(fb[:, h * 512:(h + 1) * 512], pstf[:])
                gt = gp.tile([128, S], f32, tag="gt", name="gt")
                nc.sync.dma_start(gt[:], g_in[b, :, :])
                nc.vector.tensor_tensor(out=gt[:], in0=gt[:], in1=fb[:], op=OP.add)
                mx = small.tile([128, 8], f32, tag=f"mx{b}", name=f"mx{b}")
                mi = small.tile([128, 8], u32, tag=f"mi{b}", name=f"mi{b}")
                nc.vector.max_with_indices(mx[:], mi[:], gt[:])
                idxf = small.tile([128, KF], f32, tag=f"idxf{b}", name=f"idxf{b}")
                nc.vector.tensor_copy(idxf[:], mi[:, 0:KF])
                gidx = small.tile([128, KF], i32, tag=f"gidx{b}", name=f"gidx{b}")
                nc.vector.tensor_scalar(out=gidx[:], in0=idxf[:], scalar1=float(b * S),
                                        scalar2=None, op0=OP.add)
                idxf_b.append(idxf)
                gidx_b.append(gidx)
                for j in range(KF):
                    rows = gp.tile([128, H], bf16, tag=f"grows{b}_{j}", name=f"grows{b}_{j}")
                    nc.gpsimd.indirect_dma_start(
                        out=rows[:], out_offset=None, in_=hs_bf[:],
                        in_offset=bass.IndirectOffsetOnAxis(ap=gidx[:, j:j + 1], axis=0))
                    grow_tiles[(b, j)] = rows

            def policy_chunk(c):
                cs = slice(c * 512, (c + 1) * 512)
                fp = fps.tile([1, 512], f32, space="PSUM", tag="fp", name="fp")
                for m in range(HK):
                    zp = pps.tile([128, 512], f32, space="PSUM", tag="zp", name="zp")
                    for k in range(HK):
                        nc.tensor.matmul(zp[:], wp1_sl[k][:, m * 128:(m + 1) * 128],
                                         root_bf[k][:, cs], start=(k == 0), stop=(k == HK - 1))
                    a1 = pp.tile([128, 512], bf16, tag="a1", name="a1")
                    nc.scalar.activation(a1[:], zp[:], Gelu, bias=bp1_t[:, m:m + 1])
                    nc.tensor.matmul(fp[:], wp2m[:, m:m + 1], a1[:],
                                     start=(m == 0), stop=(m == HK - 1))
                nc.scalar.activation(focus_row[:, cs], fp[:], Ident, bias=bp2m[:, :1])

            policy_chunk(0)
        # ---- sim-controller head (true fp32) ----
        logits_fm = small.tile([5, BL], f32, tag="logits")
        bsc1_row = small.tile([1, H], f32, tag="bsc1row")
        nc.sync.dma_start(bsc1_row[:], bsc1[None, :])
        with tc.tile_pool(name="scp", bufs=2) as scp, \
             tc.tile_pool(name="scsl", bufs=1) as scsl, \
             tc.tile_pool(name="scps", bufs=2, space="PSUM") as scps:
            wsc1_sl = [scsl.tile([128, H], f32, tag=f"wsc1s{k}", name=f"wsc1s{k}") for k in range(HK)]
            for k in range(HK):
                nc.sync.dma_start(wsc1_sl[k][:], Wsc1[k * 128:(k + 1) * 128, :])
            asc_tm = scp.tile([BL, H], f32, tag="asctm")
            for ch in range(2):
                chs = slice(ch * 512, (ch + 1) * 512)
                pst = scps.tile([BL, 512], f32, space="PSUM", tag="scps", name="scp1")
                for k in range(HK):
                    nc.tensor.matmul(pst[:], root0[:, k * 2:k * 2 + 2], wsc1_sl[k][:, chs],
                                     start=(k == 0), stop=False)
                nc.tensor.matmul(pst[:], ones_col[:1, :BL], bsc1_row[:, chs],
                                 start=False, stop=True)
                nc.scalar.activation(asc_tm[:, chs], pst[:], Gelu)
            asc_fm = scp.tile([128, 2 * HK], f32, tag="ascfm")
            for k in range(HK):
                pst = scps.tile([128, 128], f32, space="PSUM", tag="scps", name="scp2")
                nc.tensor.transpose(pst[:, :BL], asc_tm[:, k * 128:(k + 1) * 128], ident[:BL, :BL])
                nc.scalar.copy(asc_fm[:, k * 2:k * 2 + 2], pst[:, :BL])
            pst2 = scps.tile([5, BL], f32, space="PSUM", tag="scps", name="scp3")
            for k in range(HK):
                wsl = scsl.tile([128, 5], f32, tag="scs2")
                nc.sync.dma_start(wsl[:], Wsc2[k * 128:(k + 1) * 128, :])
                nc.tensor.matmul(pst2[:], wsl[:], asc_fm[:, k * 2:k * 2 + 2],
                                 start=(k == 0), stop=(k == HK - 1))
            nc.scalar.activation(logits_fm[:], pst2[:], Ident, bias=bsc2_t[:, :1])
        # argmax -> one-hot (via transpose + free-dim max; no partition reduce)
        lg_t = small.tile([BL, 8], f32, tag="lgt")
        nc.vector.memset(lg_t[:], -1e30)
        pst = sps.tile([128, 512], f32, space="PSUM", tag="sps")
        nc.tensor.transpose(pst[:BL, :5], logits_fm[:], ident[:5, :5])
        nc.vector.tensor_copy(lg_t[:, 0:5], pst[:BL, :5])
        rmax = small.tile([BL, 1], f32, tag="rmax")
        nc.vector.tensor_reduce(rmax[:], lg_t[:], axis=AX.X, op=OP.max)
        oh25 = small.tile([BL, 5], f32, tag="oh25")
        nc.vector.tensor_scalar(out=oh25[:], in0=lg_t[:, 0:5], scalar1=rmax[:, :1],
                                scalar2=None, op0=OP.is_equal)
        ohT = small.tile([5, BL], f32, tag="ohT")
        pst = sps.tile([128, 512], f32, space="PSUM", tag="sps")
        nc.tensor.transpose(pst[:5, :BL], oh25[:], ident[:BL, :BL])
        nc.vector.tensor_copy(ohT[:], pst[:5, :BL])
        act_ps = sps.tile([128, 512], f32, space="PSUM", tag="sps")
        for b in range(BL):
            nc.tensor.matmul(act_ps[:1, b * TP:(b + 1) * TP], ohT[:, b:b + 1], TmatT[:],
                             start=True, stop=True)
        active_row = small.tile([1, NT], f32, tag="active")
        nc.vector.tensor_copy(active_row[:], act_ps[:1, :NT])
        if DEBUG:
            nc.sync.dma_start(dbg_logits[:], logits_fm[:])
            nc.sync.dma_start(dbg_active[:], active_row[:])
        policy_chunk(1)
        b_block(0)
        policy_chunk(2)
        policy_chunk(3)
        b_block(1)
        polctx.close()

        # ---- stages 5-8 in a scope that frees before the agg stage ----
        DwF = [[late.tile([128, KF * TP], bf16, tag=f"dwf{b}_{k}", name=f"dwf{b}_{k}") for k in range(HK)]
               for b in range(BL)]
        if True:
            gathp = mid.enter_context(tc.tile_pool(name="gathp", bufs=1))
            gath = [[gathp.tile([128, NT], bf16, tag=f"gath{j}_{k}", name=f"gath{j}_{k}") for k in range(HK)]
                    for j in range(KF)]
            # stage 5: transpose pre-gathered rows -> bf16 fm (b outer: b0 first
            # so its transposes run while b1's gathers land)
            with tc.tile_pool(name="grps", bufs=4, space="PSUM") as grps:
                for b in range(BL):
                    for j in range(KF):
                        rows = grow_tiles[(b, j)]
                        for k in range(HK):
                            pst = grps.tile([128, 128], bf16, space="PSUM", tag="gtr")
                            nc.tensor.transpose(pst[:], rows[:, k * 128:(k + 1) * 128], ident_bf[:])
                            nc.scalar.copy(gath[j][k][:, b * TP:(b + 1) * TP], pst[:])

            # stage 6: transition chain j = 0,1,2 (bf16), keep diffs
            with tc.tile_pool(name="wt1p", bufs=1) as wt1p, \
                 tc.tile_pool(name="wt2p", bufs=1) as wt2p, \
                 tc.tile_pool(name="tpool", bufs=2) as tpool, \
                 tc.tile_pool(name="tps", bufs=4, space="PSUM") as tps:
                wt1_sl = [wt1p.tile([128, H], bf16, tag=f"wt1s{k}", name=f"wt1s{k}") for k in range(2 * HK)]
                for k in range(2 * HK):
                    nc.sync.dma_start(wt1_sl[k][:], Wt1[k * 128:(k + 1) * 128, :])
                wt2_sl = [wt2p.tile([128, H], bf16, tag=f"wt2s{k}", name=f"wt2s{k}") for k in range(HK)]
                for k in range(HK):
                    nc.sync.dma_start(wt2_sl[k][:], Wt2[k * 128:(k + 1) * 128, :])
                for j in range(KF):
                    at = [tpool.tile([128, NT], bf16, tag=f"at{k}", name=f"at{k}") for k in range(HK)]
                    for m in range(HK):
                        pst = tps.tile([128, NT], f32, space="PSUM", tag="tl")
                        for k in range(2 * HK):
                            rhs = mean_cur[k] if k < HK else gath[j][k - HK]
                            nc.tensor.matmul(pst[:], wt1_sl[k][:, m * 128:(m + 1) * 128], rhs[:],
                                             start=(k == 0), stop=(k == 2 * HK - 1))
                        nc.scalar.activation(at[m][:], pst[:], Gelu, bias=bt1_t[:, m:m + 1])
                    for m in range(HK):
                        pst = tps.tile([128, NT], f32, space="PSUM", tag="tl")
                        for k in range(HK):
                            nc.tensor.matmul(pst[:], wt2_sl[k][:, m * 128:(m + 1) * 128], at[k][:],
                                             start=(k == 0), stop=(k == HK - 1))
                        new_sb = tpool.tile([128, NT], f32, tag="newsb")
                        nc.scalar.activation(new_sb[:], pst[:], Ident, bias=bt2_t[:, m:m + 1])
                        dtmp = tpool.tile([128, NT], bf16, tag="dtmp")
                        nc.vector.tensor_tensor(out=dtmp[:], in0=new_sb[:],
                                                in1=gath[j][m][:], op=OP.subtract)
                        for b in range(BL):
                            nc.vector.tensor_copy(DwF[b][m][:, j * TP:(j + 1) * TP],
                                                  dtmp[:, b * TP:(b + 1) * TP])
                        sc = tpool.tile([128, NT], f32, tag="scaled")
                        nc.vector.tensor_scalar(out=sc[:], in0=dtmp[:], scalar1=1.0 / S,
                                                scalar2=None, op0=OP.mult)
                        nc.vector.tensor_tensor(out=mean_cur[m][:], in0=mean_cur[m][:],
                                                in1=sc[:], op=OP.add)

            if DEBUG:
                with tc.tile_pool(name="dbgt", bufs=1) as dbgt:
                    d1 = dbgt.tile([128, NT], f32, tag="d1", name="d1")
                    nc.vector.tensor_copy(d1[:], mean_cur[0][:])
                    nc.sync.dma_start(dbg_mean[:], d1[:])
                    d2 = dbgt.tile([128, NT], f32, tag="d2", name="d2")
                    nc.vector.tensor_copy(d2[:], gath[0][0][:])
                    nc.sync.dma_start(dbg_gath[:], d2[:])
                    d3 = dbgt.tile([128, NT], f32, tag="d3", name="d3")
                    nc.vector.tensor_copy(d3[:], DwF[0][0][:, 0:NT])
                    nc.sync.dma_start(dbg_diff[:], d3[:])

            # stage 7: value head -> sigmoid weights
            w_sig = small.tile([1, NT], f32, tag="wsig")
            with tc.tile_pool(name="wavp", bufs=1) as wavp, \
                 tc.tile_pool(name="vp", bufs=2) as vp, \
                 tc.tile_pool(name="vps", bufs=2, space="PSUM") as vps:
                wav_sl = [wavp.tile([128, H], bf16, tag=f"wavs{k}", name=f"wavs{k}") for k in range(2 * HK)]
                for k in range(2 * HK):
                    nc.sync.dma_start(wav_sl[k][:], Wav1[k * 128:(k + 1) * 128, :])
                av = [vp.tile([128, NT], bf16, tag=f"av{k}", name=f"av{k}") for k in range(HK)]
                for m in range(HK):
                    pst = vps.tile([128, NT], f32, space="PSUM", tag="vl1")
                    for k in range(2 * HK):
                        rhs = mean0[k] if k < HK else mean_cur[k - HK]
                        nc.tensor.matmul(pst[:], wav_sl[k][:, m * 128:(m + 1) * 128], rhs[:],
                                         start=(k == 0), stop=(k == 2 * HK - 1))
                    nc.scalar.activation(av[m][:], pst[:], Gelu, bias=bav1_t[:, m:m + 1])
                wps = vps.tile([1, NT], f32, space="PSUM", tag="vl2")
                wv2 = small.tile([128, HK], bf16, tag="wav2t")
                nc.sync.dma_start(wv2[:], Wav2.rearrange("(m p) o -> p (m o)", p=128))
                for k in range(HK):
                    nc.tensor.matmul(wps[:], wv2[:, k:k + 1], av[k][:],
                                     start=(k == 0), stop=(k == HK - 1))
                nc.scalar.activation(w_sig[:], wps[:], Sigm, bias=bav2_t[:, :1])

            # w~ = sigmoid * active; c = 1 + sum_t w~; broadcasts
            wt_row = small.tile([1, NT], f32, tag="wtrow")
            nc.vector.tensor_tensor(out=wt_row[:], in0=w_sig[:], in1=active_row[:], op=OP.mult)
            c_row = small.tile([1, BL], f32, tag="crow")
            nc.vector.tensor_reduce(c_row[:], wt_row[:].rearrange("p (b t) -> p b t", b=BL),
                                    axis=AX.X, op=OP.add)
            nc.vector.tensor_scalar(out=c_row[:], in0=c_row[:], scalar1=1.0, scalar2=None,
                                    op0=OP.add)
            c_tile = small.tile([128, BL], f32, tag="ctile")
            pstmp = sps.tile([128, 512], f32, space="PSUM", tag="sps")
            nc.tensor.matmul(pstmp[:, :BL], ones_col[:], c_row[:], start=True, stop=True)
            nc.vector.tensor_copy(c_tile[:], pstmp[:, :BL])
            if DEBUG:
                nc.sync.dma_start(dbg_wt[:], wt_row[:])
                nc.sync.dma_start(dbg_crow[:], c_row[:])

            # stage 8: scale DwT in place by w~; build one-hot scatter matrices
            Pmat = [[late.tile([128, S], bf16, tag=f"pm{b}_{j}", name=f"pm{b}_{j}") for j in range(KF)]
                    for b in range(BL)]
            wtcols = []
            for b in range(BL):
                wtcol = small.tile([128, 1], f32, tag=f"wtcol{b}", name=f"wtcol{b}")
                pstw = sps.tile([128, 512], f32, space="PSUM", tag="sps")
                nc.tensor.matmul(pstw[:, 0:1], wt_row[:, b * TP:(b + 1) * TP],
                                 ones_col[:, 0:1], start=True, stop=True)
                nc.vector.tensor_copy(wtcol[:], pstw[:, 0:1])
                wtcols.append(wtcol)
                for j in range(KF):
                    nc.vector.tensor_scalar(out=Pmat[b][j][:], in0=iota_f[:],
                                            scalar1=idxf_b[b][:, j:j + 1],
                                            scalar2=None, op0=OP.is_equal)
            mid.close()

        # ---- stage 9: aggregation MLP via combined weights + low-rank scatter ----
        # Z_g = [root, acc] @ Wg1 with acc = c*root + P^T Dw
        #     = root @ (Wg1a + c*Wg1b)  +  P^T (Dw @ Wg1b)
        # so agg L1 contracts over 8 k-tiles instead of 16, and the sparse part
        # enters through E = Dw @ Wg1b (rank <= 384) scattered by the one-hot P.
        with tc.tile_pool(name="gw1", bufs=1) as gw1p, \
             tc.tile_pool(name="gw2", bufs=1) as gw2p, \
             tc.tile_pool(name="wcp", bufs=1) as wcp, \
             tc.tile_pool(name="esb", bufs=1) as esbp, \
             tc.tile_pool(name="etp", bufs=2) as etp, \
             tc.tile_pool(name="egp", bufs=8) as egp, \
             tc.tile_pool(name="outp", bufs=3) as outp, \
             tc.tile_pool(name="aggps", bufs=2, space="PSUM") as aggps, \
             tc.tile_pool(name="etrps", bufs=2, space="PSUM") as etrps, \
             tc.tile_pool(name="ops", bufs=2, space="PSUM") as ops:
            wg1_sl = [gw1p.tile([128, H], bf16, tag=f"wg1s{k}", name=f"wg1s{k}") for k in range(2 * HK)]
            for k in list(range(HK, 2 * HK)) + list(range(HK)):   # Wg1b first: E path needs it
                nc.sync.dma_start(wg1_sl[k][:], Wg1[k * 128:(k + 1) * 128, :])
            wg2_sl = [gw2p.tile([128, H], bf16, tag=f"wg2s{k}", name=f"wg2s{k}") for k in range(HK)]
            for k in range(HK):
                nc.sync.dma_start(wg2_sl[k][:], Wg2[k * 128:(k + 1) * 128, :])

            # E[b] = Dw[b] @ Wg1b, transposed to [r, m] layout, scaled by w~ per row
            E_sb = [[esbp.tile([128, H], bf16, tag=f"esb{b}_{j}", name=f"esb{b}_{j}")
                     for j in range(KF)] for b in range(BL)]
            for b in range(BL):
                for m in range(HK):
                    pste = aggps.tile([128, 512], f32, space="PSUM", tag="aggps")
                    for k in range(HK):
                        nc.tensor.matmul(pste[:, :KF * TP], wg1_sl[HK + k][:, m * 128:(m + 1) * 128],
                                         DwF[b][k][:], start=(k == 0), stop=(k == HK - 1))
                    etmp = etp.tile([128, KF * TP], bf16, tag="etmp")
                    nc.scalar.copy(etmp[:], pste[:, :KF * TP])
                    for j in range(KF):
                        pstt = etrps.tile([128, 128], bf16, space="PSUM", tag="etr")
                        nc.tensor.transpose(pstt[:], etmp[:, j * TP:(j + 1) * TP], ident_bf[:])
                        nc.scalar.copy(E_sb[b][j][:, m * 128:(m + 1) * 128], pstt[:])
            for b in range(BL):
                for j in range(KF):
                    nc.vector.tensor_scalar(out=E_sb[b][j][:], in0=E_sb[b][j][:],
                                            scalar1=wtcols[b][:, :1], scalar2=None, op0=OP.mult)

            wcomb = [wcp.tile([128, H], bf16, tag=f"wc{k}", name=f"wc{k}") for k in range(HK)]
            for c in range(4):
                b = c // 2
                cs = slice(c * 512, (c + 1) * 512)            # token cols of this chunk
                ss = slice((c % 2) * 512, (c % 2 + 1) * 512)  # s cols within example b
                if c % 2 == 0:
                    for k in range(HK):
                        wtmp = etp.tile([128, H], bf16, tag="wtmp")
                        nc.vector.tensor_scalar(out=wtmp[:], in0=wg1_sl[HK + k][:],
                                                scalar1=c_tile[:, b:b + 1], scalar2=None,
                                                op0=OP.mult)
                        nc.vector.tensor_tensor(out=wcomb[k][:], in0=wg1_sl[k][:],
                                                in1=wtmp[:], op=OP.add)
                eg = [egp.tile([128, 512], bf16, tag="eg", name="eg") for _ in range(HK)]
                for m in range(HK):
                    pst = aggps.tile([128, 512], f32, space="PSUM", tag="aggps")
                    for k in range(HK):
                        nc.tensor.matmul(pst[:], wcomb[k][:, m * 128:(m + 1) * 128],
                                         root_bf[k][:, cs], start=(k == 0), stop=False)
                    for j in range(KF):
                        nc.tensor.matmul(pst[:], E_sb[b][j][:, m * 128:(m + 1) * 128],
                                         Pmat[b][j][:, ss], start=False, stop=(j == KF - 1))
                    nc.scalar.activation(eg[m][:], pst[:], Gelu, bias=bg1_t[:, m:m + 1])
                for tt in range(4):
                    trange = slice(c * 512 + tt * 128, c * 512 + (tt + 1) * 128)
                    for hc in range(2):
                        hsl = slice(hc * 512, (hc + 1) * 512)
                        pst = ops.tile([128, 512], f32, space="PSUM", tag="ops")
                        for k in range(HK):
                            nc.tensor.matmul(pst[:], eg[k][:, tt * 128:(tt + 1) * 128],
                                             wg2_sl[k][:, hsl], start=(k == 0), stop=(k == HK - 1))
                        hs_t = outp.tile([128, 512], bf16, tag="hst")
                        nc.sync.dma_start(hs_t[:], hs_bf[trange, hsl])
                        o1 = outp.tile([128, 512], f32, tag="o1")
                        nc.vector.tensor_tensor(out=o1[:], in0=pst[:], in1=bg2_bc[:, hsl],
                                                op=OP.add)
                        nc.vector.tensor_tensor(out=o1[:], in0=o1[:], in1=hs_t[:], op=OP.add)
                        nc.sync.dma_start(out[trange, hsl], o1[:])
    return nc


def _gumbel_host():
    """Exact reproduction of the reference's gumbel draws (data-independent)."""
    import jax
    cpu = jax.devices("cpu")[0]
    with jax.default_device(cpu):
        base = jax.random.key(42)
        g = np.stack([np.asarray(jax.random.gumbel(jax.random.fold_in(base, t), (B, S),
                                                   dtype=np.float32)) for t in range(T)])
    return g  # [T, B, S]


def kernel(**inputs):
    import ml_dtypes
    from concourse.bass_utils import run_bass_kernel_spmd

    if "nc" not in _CACHE:
        _CACHE["nc"] = build_nc()
    if "g" not in _CACHE:
        _CACHE["g"] = _gumbel_host()
    nc = _CACHE["nc"]
    g = _CACHE["g"]

    bf = ml_dtypes.bfloat16
    hs = np.ascontiguousarray(np.asarray(inputs["hidden_states"], dtype=np.float32))
    mask = np.ascontiguousarray(np.asarray(inputs["attention_mask"], dtype=np.int32))
    simopt = np.array([[10], [25], [50], [75], [100]], dtype=np.int32)
    f32_names = ("Wsc1", "bsc1", "Wsc2", "bsc2", "Wp2", "bp2", "bp1", "bt1", "bt2",
                 "bav1", "bav2", "bg1", "bg2")
    bf16_names = ("Wp1", "Wt1", "Wt2", "Wav1", "Wav2", "Wg1", "Wg2")
    weights = {}
    for k in f32_names:
        weights[k] = np.ascontiguousarray(np.asarray(inputs[k], dtype=np.float32))
    for k in bf16_names:
        weights[k] = np.ascontiguousarray(np.asarray(inputs[k], dtype=np.float32).astype(bf))

    weights["wp2m_in"] = weights["Wp2"].mean(axis=1)
    weights["bp2m_in"] = weights["bp2"].mean(keepdims=True)
    in_maps = []
    for c in range(NCORES):
        bs = slice(c * BL, (c + 1) * BL)
        gpad = np.zeros((BL, TP, S), dtype=np.float32)
        gpad[:, :T, :] = np.transpose(g[:, bs, :], (1, 0, 2))
        hsbf = hs[bs].reshape(TOK, H).astype(bf)
        m = {"hs": hs[bs].reshape(TOK, H),
             "hs_bf": hsbf,
             "hs_bfT": np.ascontiguousarray(hsbf.T),
             "mask": mask[bs].reshape(1, TOK),
             "g": gpad,
             "simopt": simopt}
        m.update(weights)
        in_maps.append(m)

    res = run_bass_kernel_spmd(nc, in_maps, core_ids=list(range(NCORES)))
    if DEBUG:
        _CACHE["dbg"] = res.results
    outs = [res.results[c]["out"].reshape(BL, S, H) for c in range(NCORES)]
    return np.concatenate(outs, axis=0)


# revision 40
# speedup vs baseline: 1.0270x; 1.0270x over previous
"""Trainium2 Bass kernel for nn_AdaptiveMCTSReasoner.

Self-contained: kernel(**inputs) takes FULL inputs, shards batch across 8
NeuronCores (2 examples/core), runs one Bass/Tile kernel per core, gathers.

Algorithm restructuring (validated vs reference, rel err 4.6e-7 in fp32):
  - the 100-step scan is independent given root:
      acc = (1 + sum_t w_t) * root + sum_{t,j} w_t * (new_{t,j} - root[pos_{t,j}])
    scattered at <=300 rows/example (one-hot matmul scatter).
  - transition MLPs batch over all 100 steps (padded to 128 cols/example).
  - policy L2 mean folds into a vector: focus = GELU(root@Wp1+b) @ rowmean(Wp2).
  - gumbel noise is data-independent (key 42) -> precomputed on host exactly.
Layout: activations feature-major [H on partitions, tokens on free] so weights
load as natural [h_in, h_out] lhsT tiles (no weight transposes).
Precision: bf16 matmuls for bulk stages (error budget ~5e-3 << 2e-2 gate),
fp32 for the sim-controller head (its argmax gap is 2e-4 on this data),
float32r for the aggregation second layer.
"""
import numpy as np

B, S, H = 16, 1024, 1024
T, KF = 100, 3
NCORES = 8
BL = B // NCORES          # 2 examples per core
TOK = BL * S              # 2048 tokens per core
TP = 128                  # padded step dim per example
NT = BL * TP              # 256 step-columns per core
HK = H // 128             # 8 feature tiles

_CACHE = {}
DEBUG = False


def _install_patches(mybir, TileContext, ScopedClock):
    """This walrus build allows ONE sync wait / update per instruction.
    Split excess waits onto standalone InstEventSemaphore instructions at the
    same program position on the same engine queue (semantics preserving)."""
    if getattr(TileContext, "_mcts_patched", False):
        return
    _orig_lower = TileContext._lower_ordered_insts
    counter = [0]

    def _is_async(inst):
        n = type(inst).__name__
        return n.startswith("InstDMA") or "Collective" in n

    def _mk_event(engine, waits, updates):
        counter[0] += 1
        ev = mybir.InstEventSemaphore(name=f"I-wsplit-{counter[0]}", ins=[], outs=[])
        ev.engine = engine
        ev.sync_info = mybir.SyncInfo(on_wait=list(waits), on_update=list(updates))
        return ev

    def _patched_lower(self, ordered):
        for bb_name, insts in list(ordered.items()):
            new_insts = []
            for inst in insts:
                si = inst.sync_info
                waits = list(si.on_wait) if si else []
                ups = list(si.on_update) if si else []
                changed = False
                if len(waits) > 1:
                    for w in waits[:-1]:
                        new_insts.append(_mk_event(inst.engine, [w], []))
                    waits = [waits[-1]]
                    changed = True
                post = []
                if len(ups) > 1 and not _is_async(inst):
                    for u in ups[1:]:
                        post.append(_mk_event(inst.engine, [], [u]))
                    ups = [ups[0]]
                    changed = True
                if changed:
                    inst.sync_info = mybir.SyncInfo(on_wait=waits, on_update=ups)
                new_insts.append(inst)
                new_insts.extend(post)
            ordered[bb_name] = new_insts
        return _orig_lower(self, ordered)

    def _patched_drain(self, tick_clock, wait_clock):
        drain_inst = self.nc.sync.drain()
        wait_clock.add_sem_waits(drain_inst.ins, ScopedClock({None: tick_clock.global_clock}))
        waits = list(drain_inst.ins.sync_info.on_wait)
        if len(waits) > 1:
            drain_inst.ins.sync_info = mybir.SyncInfo(on_wait=waits[:1], on_update=[])
            for i in range(1, len(waits)):
                extra = self.nc.sync.drain()
                extra.ins.sync_info = mybir.SyncInfo(on_wait=[waits[i]], on_update=[])
        self.nc.all_engine_barrier()
        popped = self.nc._tile_sem_poison_stack.pop()
        assert popped is self._sem_poison
        self.nc.clear_and_free_semaphores(list(self.sems.allocated().values()))
        self.nc.all_engine_barrier()

    TileContext._lower_ordered_insts = _patched_lower
    TileContext._drain_and_barrier = _patched_drain
    TileContext._mcts_patched = True


def build_nc():
    from contextlib import ExitStack
    from concourse import bass, mybir
    from concourse.tile import TileContext
    from concourse.vector_clock import ScopedClock
    from concourse.masks import make_identity

    _install_patches(mybir, TileContext, ScopedClock)

    f32 = mybir.dt.float32
    f32r = mybir.dt.float32r
    bf16 = mybir.dt.bfloat16
    i32 = mybir.dt.int32
    u32 = mybir.dt.uint32
    AF = mybir.ActivationFunctionType
    OP = mybir.AluOpType
    AX = mybir.AxisListType
    Gelu, Sigm, Ident = AF.Gelu, AF.Sigmoid, AF.Identity

    nc = bass.Bass()
    dp = nc.declare_dram_parameter
    hs = dp("hs", [TOK, H], f32, isOutput=False)
    hs_bf = dp("hs_bf", [TOK, H], bf16, isOutput=False)
    hs_bfT = dp("hs_bfT", [H, TOK], bf16, isOutput=False)     # host-transposed
    mask = dp("mask", [1, TOK], i32, isOutput=False)
    g_in = dp("g", [BL, TP, S], f32, isOutput=False)          # gumbel, t>=100 rows = 0
    simopt = dp("simopt", [5, 1], i32, isOutput=False)
    wp2m_in = dp("wp2m_in", [H], f32, isOutput=False)         # rowmean(Wp2) (host)
    bp2m_in = dp("bp2m_in", [1], f32, isOutput=False)         # mean(bp2) (host)        # [10,25,50,75,100]
    Wsc1 = dp("Wsc1", [H, H], f32, isOutput=False)
    bsc1 = dp("bsc1", [H], f32, isOutput=False)
    Wsc2 = dp("Wsc2", [H, 5], f32, isOutput=False)
    bsc2 = dp("bsc2", [5], f32, isOutput=False)
    Wp1 = dp("Wp1", [H, H], bf16, isOutput=False)
    bp1 = dp("bp1", [H], f32, isOutput=False)
    Wp2 = dp("Wp2", [H, H], f32, isOutput=False)
    bp2 = dp("bp2", [H], f32, isOutput=False)
    Wt1 = dp("Wt1", [2 * H, H], bf16, isOutput=False)
    bt1 = dp("bt1", [H], f32, isOutput=False)
    Wt2 = dp("Wt2", [H, H], bf16, isOutput=False)
    bt2 = dp("bt2", [H], f32, isOutput=False)
    Wav1 = dp("Wav1", [2 * H, H], bf16, isOutput=False)
    bav1 = dp("bav1", [H], f32, isOutput=False)
    Wav2 = dp("Wav2", [H, 1], bf16, isOutput=False)
    bav2 = dp("bav2", [1], f32, isOutput=False)
    Wg1 = dp("Wg1", [2 * H, H], bf16, isOutput=False)
    bg1 = dp("bg1", [H], f32, isOutput=False)
    Wg2 = dp("Wg2", [H, H], bf16, isOutput=False)
    bg2 = dp("bg2", [H], f32, isOutput=False)
    out = dp("out", [TOK, H], f32, isOutput=True)
    if DEBUG:
        dbg_root = dp("dbg_root", [128, TOK], f32, isOutput=True)
        dbg_focus = dp("dbg_focus", [1, TOK], f32, isOutput=True)
        dbg_mi = dp("dbg_mi", [BL, 128, 8], f32, isOutput=True)
        dbg_logits = dp("dbg_logits", [5, BL], f32, isOutput=True)
        dbg_active = dp("dbg_active", [1, NT], f32, isOutput=True)
        dbg_wt = dp("dbg_wt", [1, NT], f32, isOutput=True)
        dbg_crow = dp("dbg_crow", [1, BL], f32, isOutput=True)
        dbg_rm = dp("dbg_rm", [128, 2 * HK], f32, isOutput=True)
        dbg_mean = dp("dbg_mean", [128, NT], f32, isOutput=True)
        dbg_gath = dp("dbg_gath", [128, NT], f32, isOutput=True)
        dbg_diff = dp("dbg_diff", [128, NT], f32, isOutput=True)
        dbg_acc = dp("dbg_acc", [128, 512], f32, isOutput=True)
        dbg_eg = dp("dbg_eg", [128, 512], f32, isOutput=True)

    with TileContext(nc) as tc, ExitStack() as ctx:
        P_ = ctx.enter_context           # pools that live to the end
        const = P_(tc.tile_pool(name="const", bufs=1))
        persist = P_(tc.tile_pool(name="persist", bufs=1))
        bias_p = P_(tc.tile_pool(name="bias", bufs=1))
        small = P_(tc.tile_pool(name="small", bufs=1))
        late = P_(tc.tile_pool(name="late", bufs=1))          # DwT / Pmat (stages 8-9)
        sps = P_(tc.tile_pool(name="sps", bufs=2, space="PSUM"))

        # ---- constants ----
        ident = const.tile([128, 128], f32)
        make_identity(nc, ident[:])
        ones_col = const.tile([1, 128], f32)
        nc.vector.memset(ones_col[:], 1.0)
        iota_f = const.tile([128, S], f32)
        with tc.tile_pool(name="iotp", bufs=1) as iotp:
            iota_i = iotp.tile([128, S], i32, tag="iotai", name="iotai")
            nc.gpsimd.iota(iota_i[:], pattern=[[1, S]], base=0, channel_multiplier=0)
            nc.vector.tensor_copy(iota_f[:], iota_i[:])
        # TmatT[k,t] = (t < simopt[k]), [5, TP]
        iota5 = const.tile([5, TP], i32)
        nc.gpsimd.iota(iota5[:], pattern=[[1, TP]], base=0, channel_multiplier=0)
        iota5f = const.tile([5, TP], f32)
        nc.vector.tensor_copy(iota5f[:], iota5[:])
        so_t = const.tile([5, 1], i32)
        nc.sync.dma_start(so_t[:], simopt[:])
        so_f = const.tile([5, 1], f32)
        nc.vector.tensor_copy(so_f[:], so_t[:])
        TmatT = const.tile([5, TP], f32)
        nc.vector.tensor_scalar(out=TmatT[:], in0=iota5f[:], scalar1=so_f[:, :1],
                                scalar2=None, op0=OP.is_lt)
        ident_bf = const.tile([128, 128], bf16)
        nc.vector.tensor_copy(ident_bf[:], ident[:])
        zero_nt = const.tile([128, TP], f32)
        nc.vector.memset(zero_nt[:], 0.0)

        # ---- bias tiles [128, HK] (col m) per bias vector ----
        def bias_tiles(bvec, name):
            t = bias_p.tile([128, HK], f32, tag=name)
            nc.sync.dma_start(t[:], bvec.rearrange("(m p) -> p m", p=128))
            return t
        bp1_t = bias_tiles(bp1, "bp1")
        bt1_t = bias_tiles(bt1, "bt1")
        bt2_t = bias_tiles(bt2, "bt2")
        bav1_t = bias_tiles(bav1, "bav1")
        bg1_t = bias_tiles(bg1, "bg1")
        bsc2_t = bias_p.tile([5, 1], f32, tag="bsc2")
        nc.sync.dma_start(bsc2_t[:], bsc2[:, None])
        bav2_t = bias_p.tile([1, 1], f32, tag="bav2")
        nc.sync.dma_start(bav2_t[:], bav2[:, None])

        # bg2 broadcast [128, H] (bias along free dim in token-major output)
        bg2_row = small.tile([1, H], f32, tag="bg2row")
        nc.sync.dma_start(bg2_row[:], bg2[None, :])
        bg2_bc = persist.tile([128, H], f32, tag="bg2bc")
        for hc in range(2):
            pstmp = sps.tile([128, 512], f32, space="PSUM", tag="sps")
            nc.tensor.matmul(pstmp[:], ones_col[:], bg2_row[:, hc * 512:(hc + 1) * 512],
                             start=True, stop=True)
            nc.scalar.copy(bg2_bc[:, hc * 512:(hc + 1) * 512], pstmp[:])

        # ---- stage 1: root feature-major via DMA transpose (bf16) ----
        root_bf = [persist.tile([128, TOK], bf16, tag=f"rootbf{k}", name=f"rootbf{k}") for k in range(HK)]
        root0 = small.tile([128, 2 * HK], f32, tag="root0")   # col k*2+b
        with tc.tile_pool(name="r0p", bufs=2) as r0p:
            row_ts = []
            for b in range(BL):
                row_t = r0p.tile([1, H], f32, tag=f"rowt{b}", name=f"rowt{b}")
                nc.sync.dma_start(row_t[:], hs[b * S:b * S + 1, :])
                row_ts.append(row_t)
            for k in range(HK):
                nc.sync.dma_start(root_bf[k][:], hs_bfT[k * 128:(k + 1) * 128, :])
            pst0 = sps.tile([128, 512], f32, space="PSUM", tag="sps")
            for b in range(BL):
                for k in range(HK):
                    nc.tensor.matmul(pst0[:, k * 2 + b:k * 2 + b + 1],
                                     row_ts[b][:, k * 128:(k + 1) * 128],
                                     ones_col[:, 0:1], start=True, stop=True)
            nc.vector.tensor_copy(root0[:], pst0[:, :2 * HK])

        # ---- stage 2: root_mean -> mean0 broadcast tiles (bf16) ----
        rm = small.tile([128, 2 * HK], f32, tag="rm")         # col k*2+b
        for k in range(HK):
            nc.vector.tensor_reduce(rm[:, k * 2:k * 2 + 2],
                                    root_bf[k][:].rearrange("p (b s) -> p b s", b=BL),
                                    axis=AX.X, op=OP.add)
        nc.vector.tensor_scalar(out=rm[:], in0=rm[:], scalar1=1.0 / S, scalar2=None, op0=OP.mult)
        if DEBUG:
            nc.sync.dma_start(dbg_rm[:], rm[:])
        mid = ExitStack()
        meanp = mid.enter_context(tc.tile_pool(name="meanp", bufs=1))
        mean_cur = [meanp.tile([128, NT], bf16, tag=f"mean{k}", name=f"mean{k}") for k in range(HK)]
        mean0 = [meanp.tile([128, NT], bf16, tag=f"mean0{k}", name=f"mean0{k}") for k in range(HK)]
        for k in range(HK):
            for b in range(BL):
                nc.scalar.activation(mean_cur[k][:, b * TP:(b + 1) * TP], zero_nt[:],
                                     Ident, bias=rm[:, k * 2 + b:k * 2 + b + 1], scale=0.0)
        for k in range(HK):
            nc.vector.tensor_copy(mean0[k][:], mean_cur[k][:])

        # ---- stage 3: policy L1 (bf16) + focus logits ----
        focus_row = small.tile([1, TOK], f32, tag="focus")
        wp2m = small.tile([128, HK], bf16, tag="wp2m")
        wp2m_f = small.tile([128, HK], f32, tag="wp2mf")
        nc.sync.dma_start(wp2m_f[:], wp2m_in.rearrange("(m p) -> p m", p=128))
        nc.vector.tensor_copy(wp2m[:], wp2m_f[:])
        bp2m = small.tile([1, 1], f32, tag="bp2m")
        nc.sync.dma_start(bp2m[:], bp2m_in[:, None])
        idxf_b, gidx_b = [], []
        mrow = small.tile([1, TOK], f32, tag="mrow")
        with tc.tile_pool(name="mp", bufs=1) as mp:
            mrow_i = mp.tile([1, TOK], i32, tag="mrowi", name="mrowi")
            nc.sync.dma_start(mrow_i[:], mask[:])
            nc.vector.tensor_copy(mrow[:], mrow_i[:])
            nc.vector.tensor_scalar(out=mrow[:], in0=mrow[:], scalar1=0.0, scalar2=-1e9,
                                    op0=OP.is_equal, op1=OP.mult)
        gp = mid.enter_context(tc.tile_pool(name="gp", bufs=1))
        fbp = mid.enter_context(tc.tile_pool(name="fbp", bufs=2))
        polctx = ExitStack()
        pp = polctx.enter_context(tc.tile_pool(name="pp", bufs=4))
        wp1p = polctx.enter_context(tc.tile_pool(name="wp1p", bufs=1))
        pps = polctx.enter_context(tc.tile_pool(name="pps", bufs=2, space="PSUM"))
        fps = polctx.enter_context(tc.tile_pool(name="fps", bufs=2, space="PSUM"))
        if True:
            wp1_sl = [wp1p.tile([128, H], bf16, tag=f"wp1s{k}", name=f"wp1s{k}") for k in range(HK)]
            for k in range(HK):
                nc.sync.dma_start(wp1_sl[k][:], Wp1[k * 128:(k + 1) * 128, :])
            grow_tiles = {}

            def b_block(b):
                fb = fbp.tile([128, S], f32, tag="fb", name="fb")
                for h in range(2):
                    cs2 = slice(b * S + h * 512, b * S + (h + 1) * 512)
                    nc.vector.tensor_tensor(out=focus_row[:, cs2], in0=focus_row[:, cs2],
                                            in1=mrow[:, cs2], op=OP.add)
                    pstf = sps.tile([128, 512], f32, space="PSUM", tag="sps")
                    nc.tensor.matmul(pstf[:], ones_col[:], focus_row[:, cs2],
                                     start=True, stop=True)
                    nc.scalar.copy(fb[:, h * 512:(h + 1) * 512], pstf[:])
                gt = gp.tile([128, S], f32, tag="gt", name="gt")
                nc.sync.dma_start(gt[:], g_in[b, :, :])
                nc.vector.tensor_tensor(out=gt[:], in0=gt[:], in1=fb[:], op=OP.add)
                mx = small.tile([128, 8], f32, tag=f"mx{b}", name=f"mx{b}")
                mi = small.tile([128, 8], u32, tag=f"mi{b}", name=f"mi{b}")
                nc.vector.max_with_indices(mx[:], mi[:], gt[:])
                idxf = small.tile([128, KF], f32, tag=f"idxf{b}", name=f"idxf{b}")
                nc.vector.tensor_copy(idxf[:], mi[:, 0:KF])
                gidx = small.tile([128, KF], i32, tag=f"gidx{b}", name=f"gidx{b}")
                nc.vector.tensor_scalar(out=gidx[:], in0=idxf[:], scalar1=float(b * S),
                                        scalar2=None, op0=OP.add)
                idxf_b.append(idxf)
                gidx_b.append(gidx)
                for j in range(KF):
                    rows = gp.tile([128, H], bf16, tag=f"grows{b}_{j}", name=f"grows{b}_{j}")
                    nc.gpsimd.indirect_dma_start(
                        out=rows[:], out_offset=None, in_=hs_bf[:],
                        in_offset=bass.IndirectOffsetOnAxis(ap=gidx[:, j:j + 1], axis=0))
                    grow_tiles[(b, j)] = rows

            def policy_chunk(c):
                cs = slice(c * 512, (c + 1) * 512)
                fp = fps.tile([1, 512], f32, space="PSUM", tag="fp", name="fp")
                for m in range(HK):
                    zp = pps.tile([128, 512], f32, space="PSUM", tag="zp", name="zp")
                    for k in range(HK):
                        nc.tensor.matmul(zp[:], wp1_sl[k][:, m * 128:(m + 1) * 128],
                                         root_bf[k][:, cs], start=(k == 0), stop=(k == HK - 1))
                    a1 = pp.tile([128, 512], bf16, tag="a1", name="a1")
                    nc.scalar.activation(a1[:], zp[:], Gelu, bias=bp1_t[:, m:m + 1])
                    nc.tensor.matmul(fp[:], wp2m[:, m:m + 1], a1[:],
                                     start=(m == 0), stop=(m == HK - 1))
                nc.scalar.activation(focus_row[:, cs], fp[:], Ident, bias=bp2m[:, :1])

            policy_chunk(0)
        # ---- sim-controller head (true fp32) ----
        logits_fm = small.tile([5, BL], f32, tag="logits")
        bsc1_row = small.tile([1, H], f32, tag="bsc1row")
        nc.sync.dma_start(bsc1_row[:], bsc1[None, :])
        with tc.tile_pool(name="scp", bufs=2) as scp, \
             tc.tile_pool(name="scsl", bufs=1) as scsl, \
             tc.tile_pool(name="scps", bufs=2, space="PSUM") as scps:
            wsc1_sl = [scsl.tile([128, H], f32, tag=f"wsc1s{k}", name=f"wsc1s{k}") for k in range(HK)]
            for k in range(HK):
                nc.sync.dma_start(wsc1_sl[k][:], Wsc1[k * 128:(k + 1) * 128, :])
            asc_tm = scp.tile([BL, H], f32, tag="asctm")
            for ch in range(2):
                chs = slice(ch * 512, (ch + 1) * 512)
                pst = scps.tile([BL, 512], f32, space="PSUM", tag="scps", name="scp1")
                for k in range(HK):
                    nc.tensor.matmul(pst[:], root0[:, k * 2:k * 2 + 2], wsc1_sl[k][:, chs],
                                     start=(k == 0), stop=False)
                nc.tensor.matmul(pst[:], ones_col[:1, :BL], bsc1_row[:, chs],
                                 start=False, stop=True)
                nc.scalar.activation(asc_tm[:, chs], pst[:], Gelu)
            asc_fm = scp.tile([128, 2 * HK], f32, tag="ascfm")
            for k in range(HK):
                pst = scps.tile([128, 128], f32, space="PSUM", tag="scps", name="scp2")
                nc.tensor.transpose(pst[:, :BL], asc_tm[:, k * 128:(k + 1) * 128], ident[:BL, :BL])
                nc.scalar.copy(asc_fm[:, k * 2:k * 2 + 2], pst[:, :BL])
            pst2 = scps.tile([5, BL], f32, space="PSUM", tag="scps", name="scp3")
            for k in range(HK):
                wsl = scsl.tile([128, 5], f32, tag="scs2")
                nc.sync.dma_start(wsl[:], Wsc2[k * 128:(k + 1) * 128, :])
                nc.tensor.matmul(pst2[:], wsl[:], asc_fm[:, k * 2:k * 2 + 2],
                                 start=(k == 0), stop=(k == HK - 1))
            nc.scalar.activation(logits_fm[:], pst2[:], Ident, bias=bsc2_t[:, :1])
        # argmax -> one-hot (via transpose + free-dim max; no partition reduce)
        lg_t = small.tile([BL, 8], f32, tag="lgt")
        nc.vector.memset(lg_t[:], -1e30)
        pst = sps.tile([128, 512], f32, space="PSUM", tag="sps")
        nc.tensor.transpose(pst[:BL, :5], logits_fm[:], ident[:5, :5])
        nc.vector.tensor_copy(lg_t[:, 0:5], pst[:BL, :5])
        rmax = small.tile([BL, 1], f32, tag="rmax")
        nc.vector.tensor_reduce(rmax[:], lg_t[:], axis=AX.X, op=OP.max)
        oh25 = small.tile([BL, 5], f32, tag="oh25")
        nc.vector.tensor_scalar(out=oh25[:], in0=lg_t[:, 0:5], scalar1=rmax[:, :1],
                                scalar2=None, op0=OP.is_equal)
        ohT = small.tile([5, BL], f32, tag="ohT")
        pst = sps.tile([128, 512], f32, space="PSUM", tag="sps")
        nc.tensor.transpose(pst[:5, :BL], oh25[:], ident[:BL, :BL])
        nc.vector.tensor_copy(ohT[:], pst[:5, :BL])
        act_ps = sps.tile([128, 512], f32, space="PSUM", tag="sps")
        for b in range(BL):
            nc.tensor.matmul(act_ps[:1, b * TP:(b + 1) * TP], ohT[:, b:b + 1], TmatT[:],
                             start=True, stop=True)
        active_row = small.tile([1, NT], f32, tag="active")
        nc.vector.tensor_copy(active_row[:], act_ps[:1, :NT])
        if DEBUG:
            nc.sync.dma_start(dbg_logits[:], logits_fm[:])
            nc.sync.dma_start(dbg_active[:], active_row[:])
        policy_chunk(1)
        b_block(0)
        policy_chunk(2)
        policy_chunk(3)
        b_block(1)
        polctx.close()

        # ---- stages 5-8 in a scope that frees before the agg stage ----
        DwF = [[late.tile([128, KF * TP], bf16, tag=f"dwf{b}_{k}", name=f"dwf{b}_{k}") for k in range(HK)]
               for b in range(BL)]
        if True:
            gathp = mid.enter_context(tc.tile_pool(name="gathp", bufs=1))
            gath = [[gathp.tile([128, NT], bf16, tag=f"gath{j}_{k}", name=f"gath{j}_{k}") for k in range(HK)]
                    for j in range(KF)]
            # stage 5: transpose pre-gathered rows -> bf16 fm (b outer: b0 first
            # so its transposes run while b1's gathers land)
            with tc.tile_pool(name="grps", bufs=4, space="PSUM") as grps:
                for b in range(BL):
                    for j in range(KF):
                        rows = grow_tiles[(b, j)]
                        for k in range(HK):
                            pst = grps.tile([128, 128], bf16, space="PSUM", tag="gtr")
                            nc.tensor.transpose(pst[:], rows[:, k * 128:(k + 1) * 128], ident_bf[:])
                            nc.scalar.copy(gath[j][k][:, b * TP:(b + 1) * TP], pst[:])

            # stage 6: transition chain j = 0,1,2 (bf16), keep diffs
            with tc.tile_pool(name="wt1p", bufs=1) as wt1p, \
                 tc.tile_pool(name="wt2p", bufs=1) as wt2p, \
                 tc.tile_pool(name="tpool", bufs=2) as tpool, \
                 tc.tile_pool(name="tps", bufs=4, space="PSUM") as tps:
                wt1_sl = [wt1p.tile([128, H], bf16, tag=f"wt1s{k}", name=f"wt1s{k}") for k in range(2 * HK)]
                for k in range(2 * HK):
                    nc.sync.dma_start(wt1_sl[k][:], Wt1[k * 128:(k + 1) * 128, :])
                wt2_sl = [wt2p.tile([128, H], bf16, tag=f"wt2s{k}", name=f"wt2s{k}") for k in range(HK)]
                for k in range(HK):
                    nc.sync.dma_start(wt2_sl[k][:], Wt2[k * 128:(k + 1) * 128, :])
                for j in range(KF):
                    at = [tpool.tile([128, NT], bf16, tag=f"at{k}", name=f"at{k}") for k in range(HK)]
                    for m in range(HK):
                        pst = tps.tile([128, NT], f32, space="PSUM", tag="tl")
                        for k in range(2 * HK):
                            rhs = mean_cur[k] if k < HK else gath[j][k - HK]
                            nc.tensor.matmul(pst[:], wt1_sl[k][:, m * 128:(m + 1) * 128], rhs[:],
                                             start=(k == 0), stop=(k == 2 * HK - 1))
                        nc.scalar.activation(at[m][:], pst[:], Gelu, bias=bt1_t[:, m:m + 1])
                    for m in range(HK):
                        pst = tps.tile([128, NT], f32, space="PSUM", tag="tl")
                        for k in range(HK):
                            nc.tensor.matmul(pst[:], wt2_sl[k][:, m * 128:(m + 1) * 128], at[k][:],
                                             start=(k == 0), stop=(k == HK - 1))
                        new_sb = tpool.tile([128, NT], f32, tag="newsb")
                        nc.scalar.activation(new_sb[:], pst[:], Ident, bias=bt2_t[:, m:m + 1])
                        dtmp = tpool.tile([128, NT], bf16, tag="dtmp")
                        nc.vector.tensor_tensor(out=dtmp[:], in0=new_sb[:],
                                                in1=gath[j][m][:], op=OP.subtract)
                        for b in range(BL):
                            nc.vector.tensor_copy(DwF[b][m][:, j * TP:(j + 1) * TP],
                                                  dtmp[:, b * TP:(b + 1) * TP])
                        sc = tpool.tile([128, NT], f32, tag="scaled")
                        nc.vector.tensor_scalar(out=sc[:], in0=dtmp[:], scalar1=1.0 / S,
                                                scalar2=None, op0=OP.mult)
                        nc.vector.tensor_tensor(out=mean_cur[m][:], in0=mean_cur[m][:],
                                                in1=sc[:], op=OP.add)

            if DEBUG:
                with tc.tile_pool(name="dbgt", bufs=1) as dbgt:
                    d1 = dbgt.tile([128, NT], f32, tag="d1", name="d1")
                    nc.vector.tensor_copy(d1[:], mean_cur[0][:])
                    nc.sync.dma_start(dbg_mean[:], d1[:])
                    d2 = dbgt.tile([128, NT], f32, tag="d2", name="d2")
                    nc.vector.tensor_copy(d2[:], gath[0][0][:])
                    nc.sync.dma_start(dbg_gath[:], d2[:])
                    d3 = dbgt.tile([128, NT], f32, tag="d3", name="d3")
                    nc.vector.tensor_copy(d3[:], DwF[0][0][:, 0:NT])
                    nc.sync.dma_start(dbg_diff[:], d3[:])

            # stage 7: value head -> sigmoid weights
            w_sig = small.tile([1, NT], f32, tag="wsig")
            with tc.tile_pool(name="wavp", bufs=1) as wavp, \
                 tc.tile_pool(name="vp", bufs=2) as vp, \
                 tc.tile_pool(name="vps", bufs=2, space="PSUM") as vps:
                wav_sl = [wavp.tile([128, H], bf16, tag=f"wavs{k}", name=f"wavs{k}") for k in range(2 * HK)]
                for k in range(2 * HK):
                    nc.sync.dma_start(wav_sl[k][:], Wav1[k * 128:(k + 1) * 128, :])
                av = [vp.tile([128, NT], bf16, tag=f"av{k}", name=f"av{k}") for k in range(HK)]
                for m in range(HK):
                    pst = vps.tile([128, NT], f32, space="PSUM", tag="vl1")
                    for k in range(2 * HK):
                        rhs = mean0[k] if k < HK else mean_cur[k - HK]
                        nc.tensor.matmul(pst[:], wav_sl[k][:, m * 128:(m + 1) * 128], rhs[:],
                                         start=(k == 0), stop=(k == 2 * HK - 1))
                    nc.scalar.activation(av[m][:], pst[:], Gelu, bias=bav1_t[:, m:m + 1])
                wps = vps.tile([1, NT], f32, space="PSUM", tag="vl2")
                wv2 = small.tile([128, HK], bf16, tag="wav2t")
                nc.sync.dma_start(wv2[:], Wav2.rearrange("(m p) o -> p (m o)", p=128))
                for k in range(HK):
                    nc.tensor.matmul(wps[:], wv2[:, k:k + 1], av[k][:],
                                     start=(k == 0), stop=(k == HK - 1))
                nc.scalar.activation(w_sig[:], wps[:], Sigm, bias=bav2_t[:, :1])

            # w~ = sigmoid * active; c = 1 + sum_t w~; broadcasts
            wt_row = small.tile([1, NT], f32, tag="wtrow")
            nc.vector.tensor_tensor(out=wt_row[:], in0=w_sig[:], in1=active_row[:], op=OP.mult)
            c_row = small.tile([1, BL], f32, tag="crow")
            nc.vector.tensor_reduce(c_row[:], wt_row[:].rearrange("p (b t) -> p b t", b=BL),
                                    axis=AX.X, op=OP.add)
            nc.vector.tensor_scalar(out=c_row[:], in0=c_row[:], scalar1=1.0, scalar2=None,
                                    op0=OP.add)
            c_tile = small.tile([128, BL], f32, tag="ctile")
            pstmp = sps.tile([128, 512], f32, space="PSUM", tag="sps")
            nc.tensor.matmul(pstmp[:, :BL], ones_col[:], c_row[:], start=True, stop=True)
            nc.vector.tensor_copy(c_tile[:], pstmp[:, :BL])
            if DEBUG:
                nc.sync.dma_start(dbg_wt[:], wt_row[:])
                nc.sync.dma_start(dbg_crow[:], c_row[:])

            # stage 8: scale DwT in place by w~; build one-hot scatter matrices
            Pmat = [[late.tile([128, S], bf16, tag=f"pm{b}_{j}", name=f"pm{b}_{j}") for j in range(KF)]
                    for b in range(BL)]
            wtcols = []
            for b in range(BL):
                wtcol = small.tile([128, 1], f32, tag=f"wtcol{b}", name=f"wtcol{b}")
                pstw = sps.tile([128, 512], f32, space="PSUM", tag="sps")
                nc.tensor.matmul(pstw[:, 0:1], wt_row[:, b * TP:(b + 1) * TP],
                                 ones_col[:, 0:1], start=True, stop=True)
                nc.vector.tensor_copy(wtcol[:], pstw[:, 0:1])
                wtcols.append(wtcol)
                for j in range(KF):
                    nc.vector.tensor_scalar(out=Pmat[b][j][:], in0=iota_f[:],
                                            scalar1=idxf_b[b][:, j:j + 1],
                                            scalar2=None, op0=OP.is_equal)
            mid.close()

        # ---- stage 9: aggregation MLP via combined weights + low-rank scatter ----
        # Z_g = [root, acc] @ Wg1 with acc = c*root + P^T Dw
        #     = root @ (Wg1a + c*Wg1b)  +  P^T (Dw @ Wg1b)
        # so agg L1 contracts over 8 k-tiles instead of 16, and the sparse part
        # enters through E = Dw @ Wg1b (rank <= 384) scattered by the one-hot P.
        with tc.tile_pool(name="gw1", bufs=1) as gw1p, \
             tc.tile_pool(name="gw2", bufs=1) as gw2p, \
             tc.tile_pool(name="wcp", bufs=1) as wcp, \
             tc.tile_pool(name="esb", bufs=1) as esbp, \
             tc.tile_pool(name="etp", bufs=2) as etp, \
             tc.tile_pool(name="egp", bufs=8) as egp, \
             tc.tile_pool(name="outp", bufs=4) as outp, \
             tc.tile_pool(name="aggps", bufs=2, space="PSUM") as aggps, \
             tc.tile_pool(name="etrps", bufs=2, space="PSUM") as etrps, \
             tc.tile_pool(name="ops", bufs=2, space="PSUM") as ops:
            wg1_sl = [gw1p.tile([128, H], bf16, tag=f"wg1s{k}", name=f"wg1s{k}") for k in range(2 * HK)]
            for k in list(range(HK, 2 * HK)) + list(range(HK)):   # Wg1b first: E path needs it
                nc.sync.dma_start(wg1_sl[k][:], Wg1[k * 128:(k + 1) * 128, :])
            wg2_sl = [gw2p.tile([128, H], bf16, tag=f"wg2s{k}", name=f"wg2s{k}") for k in range(HK)]
            for k in range(HK):
                nc.sync.dma_start(wg2_sl[k][:], Wg2[k * 128:(k + 1) * 128, :])

            # E[b] = Dw[b] @ Wg1b, transposed to [r, m] layout, scaled by w~ per row
            E_sb = [[esbp.tile([128, H], bf16, tag=f"esb{b}_{j}", name=f"esb{b}_{j}")
                     for j in range(KF)] for b in range(BL)]
            for b in range(BL):
                for m in range(HK):
                    pste = aggps.tile([128, 512], f32, space="PSUM", tag="aggps")
                    for k in range(HK):
                        nc.tensor.matmul(pste[:, :KF * TP], wg1_sl[HK + k][:, m * 128:(m + 1) * 128],
                                         DwF[b][k][:], start=(k == 0), stop=(k == HK - 1))
                    etmp = etp.tile([128, KF * TP], bf16, tag="etmp")
                    nc.scalar.copy(etmp[:], pste[:, :KF * TP])
                    for j in range(KF):
                        pstt = etrps.tile([128, 128], bf16, space="PSUM", tag="etr")
                        nc.tensor.transpose(pstt[:], etmp[:, j * TP:(j + 1) * TP], ident_bf[:])
                        nc.scalar.copy(E_sb[b][j][:, m * 128:(m + 1) * 128], pstt[:])
            for b in range(BL):
                for j in range(KF):
                    nc.vector.tensor_scalar(out=E_sb[b][j][:], in0=E_sb[b][j][:],
                                            scalar1=wtcols[b][:, :1], scalar2=None, op0=OP.mult)

            wcomb = [wcp.tile([128, H], bf16, tag=f"wc{k}", name=f"wc{k}") for k in range(HK)]
            for c in range(4):
                b = c // 2
                cs = slice(c * 512, (c + 1) * 512)            # token cols of this chunk
                ss = slice((c % 2) * 512, (c % 2 + 1) * 512)  # s cols within example b
                if c % 2 == 0:
                    for k in range(HK):
                        wtmp = etp.tile([128, H], bf16, tag="wtmp")
                        nc.vector.tensor_scalar(out=wtmp[:], in0=wg1_sl[HK + k][:],
                                                scalar1=c_tile[:, b:b + 1], scalar2=None,
                                                op0=OP.mult)
                        nc.vector.tensor_tensor(out=wcomb[k][:], in0=wg1_sl[k][:],
                                                in1=wtmp[:], op=OP.add)
                eg = [egp.tile([128, 512], bf16, tag="eg", name="eg") for _ in range(HK)]
                for m in range(HK):
                    pst = aggps.tile([128, 512], f32, space="PSUM", tag="aggps")
                    for k in range(HK):
                        nc.tensor.matmul(pst[:], wcomb[k][:, m * 128:(m + 1) * 128],
                                         root_bf[k][:, cs], start=(k == 0), stop=False)
                    for j in range(KF):
                        nc.tensor.matmul(pst[:], E_sb[b][j][:, m * 128:(m + 1) * 128],
                                         Pmat[b][j][:, ss], start=False, stop=(j == KF - 1))
                    nc.scalar.activation(eg[m][:], pst[:], Gelu, bias=bg1_t[:, m:m + 1])
                for tt in range(4):
                    trange = slice(c * 512 + tt * 128, c * 512 + (tt + 1) * 128)
                    for hc in range(2):
                        hsl = slice(hc * 512, (hc + 1) * 512)
                        pst = ops.tile([128, 512], f32, space="PSUM", tag="ops")
                        for k in range(HK):
                            nc.tensor.matmul(pst[:], eg[k][:, tt * 128:(tt + 1) * 128],
                                             wg2_sl[k][:, hsl], start=(k == 0), stop=(k == HK - 1))
                        hs_t = outp.tile([128, 512], bf16, tag="hst")
                        nc.sync.dma_start(hs_t[:], hs_bf[trange, hsl])
                        o1 = outp.tile([128, 512], f32, tag="o1")
                        nc.vector.tensor_tensor(out=o1[:], in0=pst[:], in1=bg2_bc[:, hsl],
                                                op=OP.add)
                        nc.vector.tensor_tensor(out=o1[:], in0=o1[:], in1=hs_t[:], op=OP.add)
                        nc.sync.dma_start(out[trange, hsl], o1[:])
    return nc


def _gumbel_host():
    """Exact reproduction of the reference's gumbel draws (data-independent)."""
    import jax
    cpu = jax.devices("cpu")[0]
    with jax.default_device(cpu):
        base = jax.random.key(42)
        g = np.stack([np.asarray(jax.random.gumbel(jax.random.fold_in(base, t), (B, S),
                                                   dtype=np.float32)) for t in range(T)])
    return g  # [T, B, S]


def kernel(**inputs):
    import ml_dtypes
    from concourse.bass_utils import run_bass_kernel_spmd

    if "nc" not in _CACHE:
        _CACHE["nc"] = build_nc()
    if "g" not in _CACHE:
        _CACHE["g"] = _gumbel_host()
    nc = _CACHE["nc"]
    g = _CACHE["g"]

    bf = ml_dtypes.bfloat16
    hs = np.ascontiguousarray(np.asarray(inputs["hidden_states"], dtype=np.float32))
    mask = np.ascontiguousarray(np.asarray(inputs["attention_mask"], dtype=np.int32))
    simopt = np.array([[10], [25], [50], [75], [100]], dtype=np.int32)
    f32_names = ("Wsc1", "bsc1", "Wsc2", "bsc2", "Wp2", "bp2", "bp1", "bt1", "bt2",
                 "bav1", "bav2", "bg1", "bg2")
    bf16_names = ("Wp1", "Wt1", "Wt2", "Wav1", "Wav2", "Wg1", "Wg2")
    weights = {}
    for k in f32_names:
        weights[k] = np.ascontiguousarray(np.asarray(inputs[k], dtype=np.float32))
    for k in bf16_names:
        weights[k] = np.ascontiguousarray(np.asarray(inputs[k], dtype=np.float32).astype(bf))

    weights["wp2m_in"] = weights["Wp2"].mean(axis=1)
    weights["bp2m_in"] = weights["bp2"].mean(keepdims=True)
    in_maps = []
    for c in range(NCORES):
        bs = slice(c * BL, (c + 1) * BL)
        gpad = np.zeros((BL, TP, S), dtype=np.float32)
        gpad[:, :T, :] = np.transpose(g[:, bs, :], (1, 0, 2))
        hsbf = hs[bs].reshape(TOK, H).astype(bf)
        m = {"hs": hs[bs].reshape(TOK, H),
             "hs_bf": hsbf,
             "hs_bfT": np.ascontiguousarray(hsbf.T),
             "mask": mask[bs].reshape(1, TOK),
             "g": gpad,
             "simopt": simopt}
        m.update(weights)
        in_maps.append(m)

    res = run_bass_kernel_spmd(nc, in_maps, core_ids=list(range(NCORES)))
    if DEBUG:
        _CACHE["dbg"] = res.results
    outs = [res.results[c]["out"].reshape(BL, S, H) for c in range(NCORES)]
    return np.concatenate(outs, axis=0)
